# revision 13
# baseline (speedup 1.0000x reference)
"""HKRPQLinear Trainium2 kernel — 8-core SPMD, token-data-parallel.

Math (matches the reference nn.Module):
  x2 = x.reshape(8192, 4096)
  cw = expand(centroids, codebooks)           # (32, 4096) cluster weight rows
  dots = x2 @ cw.T                            # routing logits (fp32 on PE)
  logits = LN(dots) * ln_weight ; soft = softmax(logits)
  qmask = any(soft > .5, -1) ; cmask = any(soft > .5, 0)   # cmask is GLOBAL
  W = expand(codes, codebooks)                # (4096, 4096)
  y = (x2 @ W.T + bias) * (qmask & repeat(cmask, 128))

Sharding: tokens split 8 ways (1024/core); W/cw/bias replicated.
W and cw are expanded on the HOST (pure input prep — the codes/codebooks
gather); W ships pre-tiled (NT, 128, C, 512) bf16 so each 4 MB slice DMA
reads 32 KB contiguous per partition; cw ships in SBUF layout
(128, C*NCL) f32 for one contiguous transfer. On chip: routing (fp32
matmul — the softmax>0.5 threshold margins sit at 4e-4 in logit units,
so bf16/fp16 routing flips mask bits) then the dense bf16 GEMM with W
slices double-buffered under the accumulation. The x stream owns the
full HBM read bandwidth (W fetches strictly after it; later slices
prefetch on the Scalar engine's DMA queue during compute). qmask rides
the PSUM eviction (Vector adds bias from PSUM, Scalar multiplies the
per-token mask column). cmask needs a global OR across cores: each core
outputs its local 32-entry row; the host ORs them and zeroes masked
128-column blocks (elementwise epilogue, exact zeros) — no on-chip
AllReduce on the eviction critical path.
"""
import numpy as np
import ml_dtypes

import concourse.bass as bass
import concourse.bacc as bacc
import concourse.mybir as mybir
import concourse.tile as tile
from concourse.bass_utils import run_bass_kernel_spmd

F32 = mybir.dt.float32
BF16 = mybir.dt.bfloat16

N_CORES = 8
B, S, IN_F, OUT_F = 4, 2048, 4096, 4096
C = 32            # codebooks (K chunks of 128)
NCL = 32          # clusters
SUB = 128         # per-codebook sub-dim
CLS = 128         # cluster size
N_TOK = B * S     # 8192
M = N_TOK // N_CORES   # 1024 tokens per core
MC = M // 128     # 8 m-chunks
NT = OUT_F // 512  # 8 n-tiles of 512
EPS = 1e-5
THRESH = 0.5

_PROG = None  # compiled program cache (compile once per process)


def _body(tc, io):
    nc = tc.nc
    xhT, xlT, wTt, chT, clT, biasf, lnw, ident, y, cmrow = (
        io["xhT"], io["xlT"], io["wTt"], io["chT"], io["clT"], io["biasf"],
        io["lnw"], io["ident"], io["y"], io["cmrow"],
    )

    pconst = tc.alloc_tile_pool(name="const", bufs=1)
    px = tc.alloc_tile_pool(name="x", bufs=1)
    pxf = tc.alloc_tile_pool(name="xf", bufs=6)
    pw = tc.alloc_tile_pool(name="w", bufs=2)
    poh = tc.alloc_tile_pool(name="oh", bufs=4)
    py_pool = tc.alloc_tile_pool(name="y", bufs=4)
    ps_dots = tc.alloc_tile_pool(name="psd", bufs=1, space="PSUM")
    ps_small = tc.alloc_tile_pool(name="pss", bufs=1, space="PSUM")
    ps_y = tc.alloc_tile_pool(name="psy", bufs=5, space="PSUM")

    # ---------------- S1: constants (x-stream-critical only) ----------------
    # routing weights as bf16 hi/lo pair, host-prearranged to SBUF layout
    chT_sb = pconst.tile([128, C, NCL], BF16)
    nc.sync.dma_start(chT_sb[:], chT)
    clT_sb = pconst.tile([128, C, NCL], BF16)
    nc.sync.dma_start(clT_sb[:], clT)
    eps_col = pconst.tile([128, 1], F32)
    nc.gpsimd.memset(eps_col[:], EPS)

    w_slices = {}

    def w_fetch(nt, eng=None):
        w_sb = pw.tile([128, C, 512], BF16, tag="w")
        src = bass.AP(wTt.tensor, wTt.offset + nt * 128 * C * 512,
                      [[C * 512, 128], [512, C], [1, 512]])
        (eng or nc.sync).dma_start(w_sb[:], src)
        w_slices[nt] = w_sb

    # ---------------- S2: stream x, cast to bf16, routing matmul ----------------
    x_bf = []
    dots_ps = [ps_dots.tile([NCL, 512], F32, tag=f"dots{h}", name=f"dots_ps{h}")
               for h in range(2)]
    for c in range(C):
        # xh persists (main GEMM operand); xl is routing-only, rotates
        xh = px.tile([128, M], BF16, tag=f"xbf{c}")
        nc.sync.dma_start(xh[:], xhT[c * 128:(c + 1) * 128, :])
        xl = pxf.tile([128, M], BF16, tag="xl")
        nc.sync.dma_start(xl[:], xlT[c * 128:(c + 1) * 128, :])
        x_bf.append(xh)
        # dots = xh@ch + xh@cl + xl@ch (drops xl@cl ~ 2^-18 — verified 0
        # mask flips, 2.2x margin on this data)
        for h in range(2):
            terms = [(chT_sb, xh), (clT_sb, xh), (chT_sb, xl)]
            for ti, (cwt, xx) in enumerate(terms):
                nc.tensor.matmul(dots_ps[h][:], cwt[:, c, :],
                                 xx[:, h * 512:(h + 1) * 512],
                                 start=(c == 0 and ti == 0),
                                 stop=(c == C - 1 and ti == 2))

    # ---------------- post-stream constants + W warmup (HBM is free now) ----
    w_fetch(0)
    ident_sb = pconst.tile([128, 128], F32)
    nc.sync.dma_start(ident_sb[:], ident)
    lnw_bc = pconst.tile([128, NCL], F32)
    lsrc = bass.AP(lnw.tensor, lnw.offset, [[0, 128], [1, NCL]])
    nc.sync.dma_start(lnw_bc[:], lsrc)
    bias_bc = pconst.tile([128, OUT_F], F32)
    bsrc = bass.AP(biasf.tensor, biasf.offset, [[0, 128], [1, OUT_F]])
    nc.sync.dma_start(bias_bc[:], bsrc)
    w_fetch(1)

    # ---------------- S3: LN + softmax + masks ----------------
    dotsT_sb = pconst.tile([NCL, M], F32)
    for h in range(2):
        nc.vector.tensor_copy(dotsT_sb[:, h * 512:(h + 1) * 512], dots_ps[h][:])

    qmask = []
    mmax = pconst.tile([128, NCL], F32)
    for mc in range(MC):
        tp_ps = ps_small.tile([128, NCL], F32, tag="tpps")
        nc.tensor.transpose(tp_ps[:], dotsT_sb[:, mc * 128:(mc + 1) * 128],
                            ident_sb[0:NCL, 0:NCL])
        d = poh.tile([128, NCL], F32, tag="dots_m")
        nc.vector.tensor_copy(d[:], tp_ps[:])
        # layernorm (no bias) * ln_weight
        mu = poh.tile([128, 1], F32, tag="mu")
        nc.vector.tensor_reduce(mu[:], d[:], mybir.AxisListType.X, mybir.AluOpType.add)
        nc.scalar.mul(mu[:], mu[:], 1.0 / NCL)
        nc.vector.tensor_scalar(d[:], d[:], mu[:], None, mybir.AluOpType.subtract)
        sq = poh.tile([128, NCL], F32, tag="sq")
        nc.vector.tensor_mul(sq[:], d[:], d[:])
        ssq = poh.tile([128, 1], F32, tag="ssq")
        nc.vector.tensor_reduce(ssq[:], sq[:], mybir.AxisListType.X, mybir.AluOpType.add)
        std = poh.tile([128, 1], F32, tag="std")
        nc.scalar.activation(std[:], ssq[:], mybir.ActivationFunctionType.Sqrt,
                             bias=eps_col[:], scale=1.0 / NCL)
        rstd = poh.tile([128, 1], F32, tag="rstd")
        nc.vector.reciprocal(rstd[:], std[:])
        nc.vector.tensor_scalar(d[:], d[:], rstd[:], None, mybir.AluOpType.mult)
        nc.vector.tensor_mul(d[:], d[:], lnw_bc[:])
        # softmax > 0.5  <=>  exp(l - max) > 0.5 * sum(exp(l - max))
        nmax = poh.tile([128, 1], F32, tag="nmax")
        nc.vector.tensor_reduce(nmax[:], d[:], mybir.AxisListType.X,
                                mybir.AluOpType.max, negate=True)
        ex = poh.tile([128, NCL], F32, tag="ex")
        nc.scalar.activation(ex[:], d[:], mybir.ActivationFunctionType.Exp,
                             bias=nmax[:])
        sume = poh.tile([128, 1], F32, tag="sume")
        nc.vector.tensor_reduce(sume[:], ex[:], mybir.AxisListType.X,
                                mybir.AluOpType.add)
        nc.scalar.mul(sume[:], sume[:], THRESH)
        mgt = poh.tile([128, NCL], F32, tag="mgt")
        nc.vector.tensor_scalar(mgt[:], ex[:], sume[:], None, mybir.AluOpType.is_gt)
        qm = pconst.tile([128, 1], F32, tag=f"qm{mc}")
        nc.vector.tensor_reduce(qm[:], mgt[:], mybir.AxisListType.X,
                                mybir.AluOpType.max)
        qmask.append(qm)
        if mc == 0:
            nc.vector.tensor_copy(mmax[:], mgt[:])
        else:
            nc.vector.tensor_max(mmax[:], mmax[:], mgt[:])

    # local cmask row -> output; the global OR across cores happens on host
    cm_row = pconst.tile([1, NCL], F32)
    nc.gpsimd.tensor_reduce(cm_row[:], mmax[:], mybir.AxisListType.C,
                            mybir.AluOpType.max)
    nc.sync.dma_start(cmrow, cm_row[:])

    # ---------------- S4: main GEMM y = (x @ W.T + bias) * qmask ----------------
    for nt in range(NT):
        if nt + 2 < NT:
            w_fetch(nt + 2, eng=nc.scalar)
        w_sb = w_slices.pop(nt)
        for mc in range(MC):
            y_ps = ps_y.tile([128, 512], F32, tag="yps")
            for c in range(C):
                nc.tensor.matmul(y_ps[:], x_bf[c][:, mc * 128:(mc + 1) * 128],
                                 w_sb[:, c, :], start=(c == 0), stop=(c == C - 1))
            y_sb = py_pool.tile([128, 512], F32, tag="ysb")
            # bias add rides the PSUM read (Vector); qmask the SBUF pass (Scalar)
            nc.vector.tensor_add(y_sb[:], y_ps[:],
                                 bias_bc[:, nt * 512:(nt + 1) * 512])
            nc.scalar.mul(y_sb[:], y_sb[:], qmask[mc][:])
            nc.sync.dma_start(y[mc * 128:(mc + 1) * 128, nt * 512:(nt + 1) * 512],
                              y_sb[:])

    for p in [ps_y, ps_small, ps_dots, py_pool, poh, pw, pxf, px, pconst]:
        p.release()


def _build_program():
    nc = bacc.Bacc("TRN2", target_bir_lowering=False, debug=False,
                   num_devices=N_CORES)
    io = {}
    io["xhT"] = nc.dram_tensor("xhT", [IN_F, M], BF16, kind="ExternalInput").ap()
    io["xlT"] = nc.dram_tensor("xlT", [IN_F, M], BF16, kind="ExternalInput").ap()
    io["wTt"] = nc.dram_tensor("wTt", [NT, 128, C, 512], BF16,
                               kind="ExternalInput").ap()
    io["chT"] = nc.dram_tensor("chT", [128, C * NCL], BF16, kind="ExternalInput").ap()
    io["clT"] = nc.dram_tensor("clT", [128, C * NCL], BF16, kind="ExternalInput").ap()
    io["biasf"] = nc.dram_tensor("biasf", [1, OUT_F], F32, kind="ExternalInput").ap()
    io["lnw"] = nc.dram_tensor("lnw", [1, NCL], F32, kind="ExternalInput").ap()
    io["ident"] = nc.dram_tensor("ident", [128, 128], F32, kind="ExternalInput").ap()
    io["y"] = nc.dram_tensor("y", [M, OUT_F], F32, kind="ExternalOutput").ap()
    io["cmrow"] = nc.dram_tensor("cmrow", [1, NCL], F32, kind="ExternalOutput").ap()

    with tile.TileContext(nc) as tc:
        _body(tc, io)
    nc.compile()
    return nc


def _expand_np(codes, codebooks):
    # codes (C, N) int; codebooks (C, 256, SUB) f32 -> (C*SUB, N) = W.T
    g = codebooks[np.arange(C)[:, None], codes]        # (C, N, SUB)
    return np.ascontiguousarray(
        g.transpose(0, 2, 1).reshape(C * SUB, codes.shape[1]))


def _sbuf_layout(a_flat):
    # (IN_F, NCL) -> (128, C*NCL) so the DMA is one contiguous transfer
    return np.ascontiguousarray(
        a_flat.reshape(C, 128, NCL).transpose(1, 0, 2).reshape(128, C * NCL))


def _prep_in_maps(x, codebooks, bias, ln_weight, codes, centroids):
    bf = ml_dtypes.bfloat16
    x2 = np.ascontiguousarray(x, dtype=np.float32).reshape(N_TOK, IN_F)
    xh2 = x2.astype(bf)
    xl2 = (x2 - xh2.astype(np.float32)).astype(bf)
    cb32 = np.ascontiguousarray(codebooks, dtype=np.float32)
    wT = _expand_np(np.asarray(codes), cb32).astype(bf)                   # (IN_F, OUT_F)
    # pre-tile (NT, 128, C, 512): partition line = 32 KB contiguous per slice
    wTt = np.ascontiguousarray(
        wT.reshape(C, 128, NT, 512).transpose(2, 1, 0, 3))
    cwT_flat = _expand_np(np.asarray(centroids), cb32)                    # (IN_F, NCL)
    ch_flat = cwT_flat.astype(bf)
    cl_flat = (cwT_flat - ch_flat.astype(np.float32)).astype(bf)
    chT = _sbuf_layout(ch_flat)
    clT = _sbuf_layout(cl_flat)
    biasf = np.ascontiguousarray(bias, dtype=np.float32).reshape(1, OUT_F)
    lnw = np.ascontiguousarray(ln_weight, dtype=np.float32).reshape(1, NCL)
    ident = np.eye(128, dtype=np.float32)

    common = dict(wTt=wTt, chT=chT, clT=clT, biasf=biasf, lnw=lnw, ident=ident)
    in_maps = []
    for i in range(N_CORES):
        xhT = np.ascontiguousarray(xh2[i * M:(i + 1) * M].T)   # (4096, 1024) bf16
        xlT = np.ascontiguousarray(xl2[i * M:(i + 1) * M].T)
        in_maps.append(dict(xhT=xhT, xlT=xlT, **common))
    return in_maps


def kernel(x, codebooks, bias, ln_weight, codes, centroids, _trace=False):
    global _PROG
    if _PROG is None:
        _PROG = _build_program()
    in_maps = _prep_in_maps(x, codebooks, bias, ln_weight, codes, centroids)
    kr = run_bass_kernel_spmd(_PROG, in_maps, list(range(N_CORES)), trace=_trace)
    y = np.concatenate([np.asarray(kr.results[i]["y"]) for i in range(N_CORES)],
                       axis=0)
    # global cmask = OR over cores' local rows; zero masked 128-col blocks
    cm = np.stack([np.asarray(kr.results[i]["cmrow"]).reshape(NCL)
                   for i in range(N_CORES)]).max(axis=0)
    y[:, np.repeat(cm < 0.5, CLS)] = 0.0
    out = y.reshape(B, S, OUT_F).astype(np.float32)
    if _trace:
        return out, kr
    return out


# revision 14
# speedup vs baseline: 1.0037x; 1.0037x over previous
"""HKRPQLinear Trainium2 kernel — 8-core SPMD, token-data-parallel.

Math (matches the reference nn.Module):
  x2 = x.reshape(8192, 4096)
  cw = expand(centroids, codebooks)           # (32, 4096) cluster weight rows
  dots = x2 @ cw.T                            # routing logits (fp32 on PE)
  logits = LN(dots) * ln_weight ; soft = softmax(logits)
  qmask = any(soft > .5, -1) ; cmask = any(soft > .5, 0)   # cmask is GLOBAL
  W = expand(codes, codebooks)                # (4096, 4096)
  y = (x2 @ W.T + bias) * (qmask & repeat(cmask, 128))

Sharding: tokens split 8 ways (1024/core); W/cw/bias replicated.
W and cw are expanded on the HOST (pure input prep — the codes/codebooks
gather); W ships pre-tiled (NT, 128, C, 512) bf16 so each 4 MB slice DMA
reads 32 KB contiguous per partition; cw ships in SBUF layout
(128, C*NCL) f32 for one contiguous transfer. On chip: routing (fp32
matmul — the softmax>0.5 threshold margins sit at 4e-4 in logit units,
so bf16/fp16 routing flips mask bits) then the dense bf16 GEMM with W
slices double-buffered under the accumulation. The x stream owns the
full HBM read bandwidth (W fetches strictly after it; later slices
prefetch on the Scalar engine's DMA queue during compute). qmask rides
the PSUM eviction (Vector adds bias from PSUM, Scalar multiplies the
per-token mask column). cmask needs a global OR across cores: each core
outputs its local 32-entry row; the host ORs them and zeroes masked
128-column blocks (elementwise epilogue, exact zeros) — no on-chip
AllReduce on the eviction critical path.
"""
import numpy as np
import ml_dtypes

import concourse.bass as bass
import concourse.bacc as bacc
import concourse.mybir as mybir
import concourse.tile as tile
from concourse.bass_utils import run_bass_kernel_spmd

F32 = mybir.dt.float32
BF16 = mybir.dt.bfloat16

N_CORES = 8
B, S, IN_F, OUT_F = 4, 2048, 4096, 4096
C = 32            # codebooks (K chunks of 128)
NCL = 32          # clusters
SUB = 128         # per-codebook sub-dim
CLS = 128         # cluster size
N_TOK = B * S     # 8192
M = N_TOK // N_CORES   # 1024 tokens per core
MC = M // 128     # 8 m-chunks
NT = OUT_F // 512  # 8 n-tiles of 512
EPS = 1e-5
THRESH = 0.5

_PROG = None  # compiled program cache (compile once per process)


def _body(tc, io):
    nc = tc.nc
    xhT, xlT, wTt, chT, clT, biasf, lnw, ident, y, cmrow = (
        io["xhT"], io["xlT"], io["wTt"], io["chT"], io["clT"], io["biasf"],
        io["lnw"], io["ident"], io["y"], io["cmrow"],
    )

    pconst = tc.alloc_tile_pool(name="const", bufs=1)
    px = tc.alloc_tile_pool(name="x", bufs=1)
    pxf = tc.alloc_tile_pool(name="xf", bufs=6)
    pw = tc.alloc_tile_pool(name="w", bufs=2)
    poh = tc.alloc_tile_pool(name="oh", bufs=4)
    py_pool = tc.alloc_tile_pool(name="y", bufs=4)
    ps_dots = tc.alloc_tile_pool(name="psd", bufs=1, space="PSUM")
    ps_small = tc.alloc_tile_pool(name="pss", bufs=1, space="PSUM")
    ps_y = tc.alloc_tile_pool(name="psy", bufs=5, space="PSUM")

    # ---------------- S1: constants (x-stream-critical only) ----------------
    # routing weights as bf16 hi/lo pair, host-prearranged to SBUF layout
    chT_sb = pconst.tile([128, C, NCL], BF16)
    nc.sync.dma_start(chT_sb[:], chT)
    clT_sb = pconst.tile([128, C, NCL], BF16)
    nc.sync.dma_start(clT_sb[:], clT)
    eps_col = pconst.tile([128, 1], F32)
    nc.gpsimd.memset(eps_col[:], EPS)

    w_slices = {}

    def w_fetch(nt, eng=None):
        w_sb = pw.tile([128, C, 512], BF16, tag="w")
        src = bass.AP(wTt.tensor, wTt.offset + nt * 128 * C * 512,
                      [[C * 512, 128], [512, C], [1, 512]])
        (eng or nc.sync).dma_start(w_sb[:], src)
        w_slices[nt] = w_sb

    # ---------------- S2: stream x, cast to bf16, routing matmul ----------------
    x_bf = []
    dots_ps = [ps_dots.tile([NCL, 512], F32, tag=f"dots{h}", name=f"dots_ps{h}")
               for h in range(2)]
    for c in range(C):
        # xh persists (main GEMM operand); xl is routing-only, rotates
        xh = px.tile([128, M], BF16, tag=f"xbf{c}")
        nc.sync.dma_start(xh[:], xhT[c * 128:(c + 1) * 128, :])
        xl = pxf.tile([128, M], BF16, tag="xl")
        # tail xl chunks drain on the Scalar queue so the Sync queue reaches
        # the W0 fetch sooner (xl feeds only the routing epilogue); emitted
        # in-loop so writer-before-reader dependency order is preserved
        xl_eng = nc.scalar if c >= C - 6 else nc.sync
        xl_eng.dma_start(xl[:], xlT[c * 128:(c + 1) * 128, :])
        x_bf.append(xh)
        # dots = xh@ch + xh@cl + xl@ch (drops xl@cl ~ 2^-18 — verified 0
        # mask flips, 2.2x margin on this data)
        for h in range(2):
            terms = [(chT_sb, xh), (clT_sb, xh), (chT_sb, xl)]
            for ti, (cwt, xx) in enumerate(terms):
                nc.tensor.matmul(dots_ps[h][:], cwt[:, c, :],
                                 xx[:, h * 512:(h + 1) * 512],
                                 start=(c == 0 and ti == 0),
                                 stop=(c == C - 1 and ti == 2))

    # ---------------- post-stream constants + W warmup (HBM is free now) ----
    w_fetch(0)
    ident_sb = pconst.tile([128, 128], F32)
    nc.sync.dma_start(ident_sb[:], ident)
    lnw_bc = pconst.tile([128, NCL], F32)
    lsrc = bass.AP(lnw.tensor, lnw.offset, [[0, 128], [1, NCL]])
    nc.sync.dma_start(lnw_bc[:], lsrc)
    bias_bc = pconst.tile([128, OUT_F], F32)
    bsrc = bass.AP(biasf.tensor, biasf.offset, [[0, 128], [1, OUT_F]])
    nc.sync.dma_start(bias_bc[:], bsrc)
    w_fetch(1)

    # ---------------- S3: LN + softmax + masks ----------------
    dotsT_sb = pconst.tile([NCL, M], F32)
    for h in range(2):
        nc.vector.tensor_copy(dotsT_sb[:, h * 512:(h + 1) * 512], dots_ps[h][:])

    qmask = []
    mmax = pconst.tile([128, NCL], F32)
    for mc in range(MC):
        tp_ps = ps_small.tile([128, NCL], F32, tag="tpps")
        nc.tensor.transpose(tp_ps[:], dotsT_sb[:, mc * 128:(mc + 1) * 128],
                            ident_sb[0:NCL, 0:NCL])
        d = poh.tile([128, NCL], F32, tag="dots_m")
        nc.vector.tensor_copy(d[:], tp_ps[:])
        # layernorm (no bias) * ln_weight
        mu = poh.tile([128, 1], F32, tag="mu")
        nc.vector.tensor_reduce(mu[:], d[:], mybir.AxisListType.X, mybir.AluOpType.add)
        nc.scalar.mul(mu[:], mu[:], 1.0 / NCL)
        nc.vector.tensor_scalar(d[:], d[:], mu[:], None, mybir.AluOpType.subtract)
        sq = poh.tile([128, NCL], F32, tag="sq")
        nc.vector.tensor_mul(sq[:], d[:], d[:])
        ssq = poh.tile([128, 1], F32, tag="ssq")
        nc.vector.tensor_reduce(ssq[:], sq[:], mybir.AxisListType.X, mybir.AluOpType.add)
        std = poh.tile([128, 1], F32, tag="std")
        nc.scalar.activation(std[:], ssq[:], mybir.ActivationFunctionType.Sqrt,
                             bias=eps_col[:], scale=1.0 / NCL)
        rstd = poh.tile([128, 1], F32, tag="rstd")
        nc.vector.reciprocal(rstd[:], std[:])
        nc.vector.tensor_scalar(d[:], d[:], rstd[:], None, mybir.AluOpType.mult)
        nc.vector.tensor_mul(d[:], d[:], lnw_bc[:])
        # softmax > 0.5  <=>  exp(l - max) > 0.5 * sum(exp(l - max))
        nmax = poh.tile([128, 1], F32, tag="nmax")
        nc.vector.tensor_reduce(nmax[:], d[:], mybir.AxisListType.X,
                                mybir.AluOpType.max, negate=True)
        ex = poh.tile([128, NCL], F32, tag="ex")
        nc.scalar.activation(ex[:], d[:], mybir.ActivationFunctionType.Exp,
                             bias=nmax[:])
        sume = poh.tile([128, 1], F32, tag="sume")
        nc.vector.tensor_reduce(sume[:], ex[:], mybir.AxisListType.X,
                                mybir.AluOpType.add)
        nc.scalar.mul(sume[:], sume[:], THRESH)
        mgt = poh.tile([128, NCL], F32, tag="mgt")
        nc.vector.tensor_scalar(mgt[:], ex[:], sume[:], None, mybir.AluOpType.is_gt)
        qm = pconst.tile([128, 1], F32, tag=f"qm{mc}")
        nc.vector.tensor_reduce(qm[:], mgt[:], mybir.AxisListType.X,
                                mybir.AluOpType.max)
        qmask.append(qm)
        if mc == 0:
            nc.vector.tensor_copy(mmax[:], mgt[:])
        else:
            nc.vector.tensor_max(mmax[:], mmax[:], mgt[:])

    # local cmask row -> output; the global OR across cores happens on host
    cm_row = pconst.tile([1, NCL], F32)
    nc.gpsimd.tensor_reduce(cm_row[:], mmax[:], mybir.AxisListType.C,
                            mybir.AluOpType.max)
    nc.sync.dma_start(cmrow, cm_row[:])

    # ---------------- S4: main GEMM y = (x @ W.T + bias) * qmask ----------------
    for nt in range(NT):
        if nt + 2 < NT:
            w_fetch(nt + 2, eng=nc.scalar)
        w_sb = w_slices.pop(nt)
        for mc in range(MC):
            y_ps = ps_y.tile([128, 512], F32, tag="yps")
            for c in range(C):
                nc.tensor.matmul(y_ps[:], x_bf[c][:, mc * 128:(mc + 1) * 128],
                                 w_sb[:, c, :], start=(c == 0), stop=(c == C - 1))
            y_sb = py_pool.tile([128, 512], F32, tag="ysb")
            # bias add rides the PSUM read (Vector); qmask the SBUF pass (Scalar)
            nc.vector.tensor_add(y_sb[:], y_ps[:],
                                 bias_bc[:, nt * 512:(nt + 1) * 512])
            nc.scalar.mul(y_sb[:], y_sb[:], qmask[mc][:])
            nc.sync.dma_start(y[mc * 128:(mc + 1) * 128, nt * 512:(nt + 1) * 512],
                              y_sb[:])

    for p in [ps_y, ps_small, ps_dots, py_pool, poh, pw, pxf, px, pconst]:
        p.release()


def _build_program():
    nc = bacc.Bacc("TRN2", target_bir_lowering=False, debug=False,
                   num_devices=N_CORES)
    io = {}
    io["xhT"] = nc.dram_tensor("xhT", [IN_F, M], BF16, kind="ExternalInput").ap()
    io["xlT"] = nc.dram_tensor("xlT", [IN_F, M], BF16, kind="ExternalInput").ap()
    io["wTt"] = nc.dram_tensor("wTt", [NT, 128, C, 512], BF16,
                               kind="ExternalInput").ap()
    io["chT"] = nc.dram_tensor("chT", [128, C * NCL], BF16, kind="ExternalInput").ap()
    io["clT"] = nc.dram_tensor("clT", [128, C * NCL], BF16, kind="ExternalInput").ap()
    io["biasf"] = nc.dram_tensor("biasf", [1, OUT_F], F32, kind="ExternalInput").ap()
    io["lnw"] = nc.dram_tensor("lnw", [1, NCL], F32, kind="ExternalInput").ap()
    io["ident"] = nc.dram_tensor("ident", [128, 128], F32, kind="ExternalInput").ap()
    io["y"] = nc.dram_tensor("y", [M, OUT_F], F32, kind="ExternalOutput").ap()
    io["cmrow"] = nc.dram_tensor("cmrow", [1, NCL], F32, kind="ExternalOutput").ap()

    with tile.TileContext(nc) as tc:
        _body(tc, io)
    nc.compile()
    return nc


def _expand_np(codes, codebooks):
    # codes (C, N) int; codebooks (C, 256, SUB) f32 -> (C*SUB, N) = W.T
    g = codebooks[np.arange(C)[:, None], codes]        # (C, N, SUB)
    return np.ascontiguousarray(
        g.transpose(0, 2, 1).reshape(C * SUB, codes.shape[1]))


def _sbuf_layout(a_flat):
    # (IN_F, NCL) -> (128, C*NCL) so the DMA is one contiguous transfer
    return np.ascontiguousarray(
        a_flat.reshape(C, 128, NCL).transpose(1, 0, 2).reshape(128, C * NCL))


def _prep_in_maps(x, codebooks, bias, ln_weight, codes, centroids):
    bf = ml_dtypes.bfloat16
    x2 = np.ascontiguousarray(x, dtype=np.float32).reshape(N_TOK, IN_F)
    xh2 = x2.astype(bf)
    xl2 = (x2 - xh2.astype(np.float32)).astype(bf)
    cb32 = np.ascontiguousarray(codebooks, dtype=np.float32)
    wT = _expand_np(np.asarray(codes), cb32).astype(bf)                   # (IN_F, OUT_F)
    # pre-tile (NT, 128, C, 512): partition line = 32 KB contiguous per slice
    wTt = np.ascontiguousarray(
        wT.reshape(C, 128, NT, 512).transpose(2, 1, 0, 3))
    cwT_flat = _expand_np(np.asarray(centroids), cb32)                    # (IN_F, NCL)
    ch_flat = cwT_flat.astype(bf)
    cl_flat = (cwT_flat - ch_flat.astype(np.float32)).astype(bf)
    chT = _sbuf_layout(ch_flat)
    clT = _sbuf_layout(cl_flat)
    biasf = np.ascontiguousarray(bias, dtype=np.float32).reshape(1, OUT_F)
    lnw = np.ascontiguousarray(ln_weight, dtype=np.float32).reshape(1, NCL)
    ident = np.eye(128, dtype=np.float32)

    common = dict(wTt=wTt, chT=chT, clT=clT, biasf=biasf, lnw=lnw, ident=ident)
    in_maps = []
    for i in range(N_CORES):
        xhT = np.ascontiguousarray(xh2[i * M:(i + 1) * M].T)   # (4096, 1024) bf16
        xlT = np.ascontiguousarray(xl2[i * M:(i + 1) * M].T)
        in_maps.append(dict(xhT=xhT, xlT=xlT, **common))
    return in_maps


def kernel(x, codebooks, bias, ln_weight, codes, centroids, _trace=False):
    global _PROG
    if _PROG is None:
        _PROG = _build_program()
    in_maps = _prep_in_maps(x, codebooks, bias, ln_weight, codes, centroids)
    kr = run_bass_kernel_spmd(_PROG, in_maps, list(range(N_CORES)), trace=_trace)
    y = np.concatenate([np.asarray(kr.results[i]["y"]) for i in range(N_CORES)],
                       axis=0)
    # global cmask = OR over cores' local rows; zero masked 128-col blocks
    cm = np.stack([np.asarray(kr.results[i]["cmrow"]).reshape(NCL)
                   for i in range(N_CORES)]).max(axis=0)
    y[:, np.repeat(cm < 0.5, CLS)] = 0.0
    out = y.reshape(B, S, OUT_F).astype(np.float32)
    if _trace:
        return out, kr
    return out


# revision 15
# speedup vs baseline: 1.0247x; 1.0209x over previous
"""HKRPQLinear Trainium2 kernel — 8-core SPMD, token-data-parallel.

Math (matches the reference nn.Module):
  x2 = x.reshape(8192, 4096)
  cw = expand(centroids, codebooks)           # (32, 4096) cluster weight rows
  dots = x2 @ cw.T                            # routing logits (fp32 on PE)
  logits = LN(dots) * ln_weight ; soft = softmax(logits)
  qmask = any(soft > .5, -1) ; cmask = any(soft > .5, 0)   # cmask is GLOBAL
  W = expand(codes, codebooks)                # (4096, 4096)
  y = (x2 @ W.T + bias) * (qmask & repeat(cmask, 128))

Sharding: tokens split 8 ways (1024/core); W/cw/bias replicated.
W and cw are expanded on the HOST (pure input prep — the codes/codebooks
gather); W ships pre-tiled (NT, 128, C, 512) bf16 so each 4 MB slice DMA
reads 32 KB contiguous per partition; cw ships in SBUF layout
(128, C*NCL) f32 for one contiguous transfer. On chip: routing (fp32
matmul — the softmax>0.5 threshold margins sit at 4e-4 in logit units,
so bf16/fp16 routing flips mask bits) then the dense bf16 GEMM with W
slices double-buffered under the accumulation. The x stream owns the
full HBM read bandwidth (W fetches strictly after it; later slices
prefetch on the Scalar engine's DMA queue during compute). qmask rides
the PSUM eviction (Vector adds bias from PSUM, Scalar multiplies the
per-token mask column). cmask needs a global OR across cores: each core
outputs its local 32-entry row; the host ORs them and zeroes masked
128-column blocks (elementwise epilogue, exact zeros) — no on-chip
AllReduce on the eviction critical path.
"""
import numpy as np
import ml_dtypes

import concourse.bass as bass
import concourse.bacc as bacc
import concourse.mybir as mybir
import concourse.tile as tile
from concourse.bass_utils import run_bass_kernel_spmd

F32 = mybir.dt.float32
BF16 = mybir.dt.bfloat16

N_CORES = 8
B, S, IN_F, OUT_F = 4, 2048, 4096, 4096
C = 32            # codebooks (K chunks of 128)
NCL = 32          # clusters
SUB = 128         # per-codebook sub-dim
CLS = 128         # cluster size
N_TOK = B * S     # 8192
M = N_TOK // N_CORES   # 1024 tokens per core
MC = M // 128     # 8 m-chunks
NT = OUT_F // 512  # 8 n-tiles of 512
EPS = 1e-5
THRESH = 0.5

_PROG = None  # compiled program cache (compile once per process)


def _body(tc, io):
    nc = tc.nc
    xhT, xlT, wTt, chT, clT, biasf, lnw, ident, y, cmrow = (
        io["xhT"], io["xlT"], io["wTt"], io["chT"], io["clT"], io["biasf"],
        io["lnw"], io["ident"], io["y"], io["cmrow"],
    )

    pconst = tc.alloc_tile_pool(name="const", bufs=1)
    px = tc.alloc_tile_pool(name="x", bufs=1)
    pxf = tc.alloc_tile_pool(name="xf", bufs=6)
    pw = tc.alloc_tile_pool(name="w", bufs=2)
    poh = tc.alloc_tile_pool(name="oh", bufs=4)
    py_pool = tc.alloc_tile_pool(name="y", bufs=4)
    ps_dots = tc.alloc_tile_pool(name="psd", bufs=1, space="PSUM")
    ps_small = tc.alloc_tile_pool(name="pss", bufs=1, space="PSUM")
    ps_y = tc.alloc_tile_pool(name="psy", bufs=5, space="PSUM")

    # ---------------- S1: constants (x-stream-critical only) ----------------
    # routing weights as bf16 hi/lo pair, host-prearranged to SBUF layout
    chT_sb = pconst.tile([128, C, NCL], BF16)
    nc.sync.dma_start(chT_sb[:], chT)
    clT_sb = pconst.tile([128, C, NCL], BF16)
    nc.sync.dma_start(clT_sb[:], clT)
    eps_col = pconst.tile([128, 1], F32)
    nc.gpsimd.memset(eps_col[:], EPS)

    w_slices = {}

    def w_fetch(nt, eng=None, parts=1):
        # parts>1 splits the slice into c-range quarter DMAs filling one tile;
        # subtile deps let the first matmuls start after the first quarter
        w_sb = pw.tile([128, C, 512], BF16, tag="w")
        cn = C // parts
        for p in range(parts):
            src = bass.AP(wTt.tensor,
                          wTt.offset + nt * 128 * C * 512 + p * cn * 512,
                          [[C * 512, 128], [512, cn], [1, 512]])
            (eng or nc.sync).dma_start(w_sb[:, p * cn:(p + 1) * cn, :], src)
        w_slices[nt] = w_sb

    # ---------------- S2: stream x, cast to bf16, routing matmul ----------------
    x_bf = []
    dots_ps = [ps_dots.tile([NCL, 512], F32, tag=f"dots{h}", name=f"dots_ps{h}")
               for h in range(2)]
    for c in range(C):
        # xh persists (main GEMM operand); xl is routing-only, rotates
        xh = px.tile([128, M], BF16, tag=f"xbf{c}")
        nc.sync.dma_start(xh[:], xhT[c * 128:(c + 1) * 128, :])
        xl = pxf.tile([128, M], BF16, tag="xl")
        # tail xl chunks drain on the Scalar queue so the Sync queue reaches
        # the W0 fetch sooner (xl feeds only the routing epilogue); emitted
        # in-loop so writer-before-reader dependency order is preserved
        xl_eng = nc.scalar if c >= C - 6 else nc.sync
        xl_eng.dma_start(xl[:], xlT[c * 128:(c + 1) * 128, :])
        x_bf.append(xh)
        # dots = xh@ch + xh@cl + xl@ch (drops xl@cl ~ 2^-18 — verified 0
        # mask flips, 2.2x margin on this data)
        for h in range(2):
            terms = [(chT_sb, xh), (clT_sb, xh), (chT_sb, xl)]
            for ti, (cwt, xx) in enumerate(terms):
                nc.tensor.matmul(dots_ps[h][:], cwt[:, c, :],
                                 xx[:, h * 512:(h + 1) * 512],
                                 start=(c == 0 and ti == 0),
                                 stop=(c == C - 1 and ti == 2))

    # ---------------- post-stream constants + W warmup (HBM is free now) ----
    w_fetch(0, parts=4)
    ident_sb = pconst.tile([128, 128], F32)
    nc.sync.dma_start(ident_sb[:], ident)
    lnw_bc = pconst.tile([128, NCL], F32)
    lsrc = bass.AP(lnw.tensor, lnw.offset, [[0, 128], [1, NCL]])
    nc.sync.dma_start(lnw_bc[:], lsrc)
    bias_bc = pconst.tile([128, OUT_F], F32)
    bsrc = bass.AP(biasf.tensor, biasf.offset, [[0, 128], [1, OUT_F]])
    nc.sync.dma_start(bias_bc[:], bsrc)
    w_fetch(1)

    # ---------------- S3: LN + softmax + masks ----------------
    dotsT_sb = pconst.tile([NCL, M], F32)
    for h in range(2):
        nc.vector.tensor_copy(dotsT_sb[:, h * 512:(h + 1) * 512], dots_ps[h][:])

    qmask = []
    mmax = pconst.tile([128, NCL], F32)
    for mc in range(MC):
        tp_ps = ps_small.tile([128, NCL], F32, tag="tpps")
        nc.tensor.transpose(tp_ps[:], dotsT_sb[:, mc * 128:(mc + 1) * 128],
                            ident_sb[0:NCL, 0:NCL])
        d = poh.tile([128, NCL], F32, tag="dots_m")
        nc.vector.tensor_copy(d[:], tp_ps[:])
        # layernorm (no bias) * ln_weight
        mu = poh.tile([128, 1], F32, tag="mu")
        nc.vector.tensor_reduce(mu[:], d[:], mybir.AxisListType.X, mybir.AluOpType.add)
        nc.scalar.mul(mu[:], mu[:], 1.0 / NCL)
        nc.vector.tensor_scalar(d[:], d[:], mu[:], None, mybir.AluOpType.subtract)
        sq = poh.tile([128, NCL], F32, tag="sq")
        nc.vector.tensor_mul(sq[:], d[:], d[:])
        ssq = poh.tile([128, 1], F32, tag="ssq")
        nc.vector.tensor_reduce(ssq[:], sq[:], mybir.AxisListType.X, mybir.AluOpType.add)
        std = poh.tile([128, 1], F32, tag="std")
        nc.scalar.activation(std[:], ssq[:], mybir.ActivationFunctionType.Sqrt,
                             bias=eps_col[:], scale=1.0 / NCL)
        rstd = poh.tile([128, 1], F32, tag="rstd")
        nc.vector.reciprocal(rstd[:], std[:])
        nc.vector.tensor_scalar(d[:], d[:], rstd[:], None, mybir.AluOpType.mult)
        nc.vector.tensor_mul(d[:], d[:], lnw_bc[:])
        # softmax > 0.5  <=>  exp(l - max) > 0.5 * sum(exp(l - max))
        nmax = poh.tile([128, 1], F32, tag="nmax")
        nc.vector.tensor_reduce(nmax[:], d[:], mybir.AxisListType.X,
                                mybir.AluOpType.max, negate=True)
        ex = poh.tile([128, NCL], F32, tag="ex")
        nc.scalar.activation(ex[:], d[:], mybir.ActivationFunctionType.Exp,
                             bias=nmax[:])
        sume = poh.tile([128, 1], F32, tag="sume")
        nc.vector.tensor_reduce(sume[:], ex[:], mybir.AxisListType.X,
                                mybir.AluOpType.add)
        nc.scalar.mul(sume[:], sume[:], THRESH)
        mgt = poh.tile([128, NCL], F32, tag="mgt")
        nc.vector.tensor_scalar(mgt[:], ex[:], sume[:], None, mybir.AluOpType.is_gt)
        qm = pconst.tile([128, 1], F32, tag=f"qm{mc}")
        nc.vector.tensor_reduce(qm[:], mgt[:], mybir.AxisListType.X,
                                mybir.AluOpType.max)
        qmask.append(qm)
        if mc == 0:
            nc.vector.tensor_copy(mmax[:], mgt[:])
        else:
            nc.vector.tensor_max(mmax[:], mmax[:], mgt[:])

    # local cmask row -> output; the global OR across cores happens on host
    cm_row = pconst.tile([1, NCL], F32)
    nc.gpsimd.tensor_reduce(cm_row[:], mmax[:], mybir.AxisListType.C,
                            mybir.AluOpType.max)
    nc.sync.dma_start(cmrow, cm_row[:])

    # ---------------- S4: main GEMM y = (x @ W.T + bias) * qmask ----------------
    for nt in range(NT):
        if nt + 2 < NT:
            w_fetch(nt + 2, eng=nc.scalar)
        w_sb = w_slices.pop(nt)
        for mc in range(MC):
            y_ps = ps_y.tile([128, 512], F32, tag="yps")
            for c in range(C):
                nc.tensor.matmul(y_ps[:], x_bf[c][:, mc * 128:(mc + 1) * 128],
                                 w_sb[:, c, :], start=(c == 0), stop=(c == C - 1))
            y_sb = py_pool.tile([128, 512], F32, tag="ysb")
            # bias add rides the PSUM read (Vector); qmask the SBUF pass (Scalar)
            nc.vector.tensor_add(y_sb[:], y_ps[:],
                                 bias_bc[:, nt * 512:(nt + 1) * 512])
            nc.scalar.mul(y_sb[:], y_sb[:], qmask[mc][:])
            nc.sync.dma_start(y[mc * 128:(mc + 1) * 128, nt * 512:(nt + 1) * 512],
                              y_sb[:])

    for p in [ps_y, ps_small, ps_dots, py_pool, poh, pw, pxf, px, pconst]:
        p.release()


def _build_program():
    nc = bacc.Bacc("TRN2", target_bir_lowering=False, debug=False,
                   num_devices=N_CORES)
    io = {}
    io["xhT"] = nc.dram_tensor("xhT", [IN_F, M], BF16, kind="ExternalInput").ap()
    io["xlT"] = nc.dram_tensor("xlT", [IN_F, M], BF16, kind="ExternalInput").ap()
    io["wTt"] = nc.dram_tensor("wTt", [NT, 128, C, 512], BF16,
                               kind="ExternalInput").ap()
    io["chT"] = nc.dram_tensor("chT", [128, C * NCL], BF16, kind="ExternalInput").ap()
    io["clT"] = nc.dram_tensor("clT", [128, C * NCL], BF16, kind="ExternalInput").ap()
    io["biasf"] = nc.dram_tensor("biasf", [1, OUT_F], F32, kind="ExternalInput").ap()
    io["lnw"] = nc.dram_tensor("lnw", [1, NCL], F32, kind="ExternalInput").ap()
    io["ident"] = nc.dram_tensor("ident", [128, 128], F32, kind="ExternalInput").ap()
    io["y"] = nc.dram_tensor("y", [M, OUT_F], F32, kind="ExternalOutput").ap()
    io["cmrow"] = nc.dram_tensor("cmrow", [1, NCL], F32, kind="ExternalOutput").ap()

    with tile.TileContext(nc) as tc:
        _body(tc, io)
    nc.compile()
    return nc


def _expand_np(codes, codebooks):
    # codes (C, N) int; codebooks (C, 256, SUB) f32 -> (C*SUB, N) = W.T
    g = codebooks[np.arange(C)[:, None], codes]        # (C, N, SUB)
    return np.ascontiguousarray(
        g.transpose(0, 2, 1).reshape(C * SUB, codes.shape[1]))


def _sbuf_layout(a_flat):
    # (IN_F, NCL) -> (128, C*NCL) so the DMA is one contiguous transfer
    return np.ascontiguousarray(
        a_flat.reshape(C, 128, NCL).transpose(1, 0, 2).reshape(128, C * NCL))


def _prep_in_maps(x, codebooks, bias, ln_weight, codes, centroids):
    bf = ml_dtypes.bfloat16
    x2 = np.ascontiguousarray(x, dtype=np.float32).reshape(N_TOK, IN_F)
    xh2 = x2.astype(bf)
    xl2 = (x2 - xh2.astype(np.float32)).astype(bf)
    cb32 = np.ascontiguousarray(codebooks, dtype=np.float32)
    wT = _expand_np(np.asarray(codes), cb32).astype(bf)                   # (IN_F, OUT_F)
    # pre-tile (NT, 128, C, 512): partition line = 32 KB contiguous per slice
    wTt = np.ascontiguousarray(
        wT.reshape(C, 128, NT, 512).transpose(2, 1, 0, 3))
    cwT_flat = _expand_np(np.asarray(centroids), cb32)                    # (IN_F, NCL)
    ch_flat = cwT_flat.astype(bf)
    cl_flat = (cwT_flat - ch_flat.astype(np.float32)).astype(bf)
    chT = _sbuf_layout(ch_flat)
    clT = _sbuf_layout(cl_flat)
    biasf = np.ascontiguousarray(bias, dtype=np.float32).reshape(1, OUT_F)
    lnw = np.ascontiguousarray(ln_weight, dtype=np.float32).reshape(1, NCL)
    ident = np.eye(128, dtype=np.float32)

    common = dict(wTt=wTt, chT=chT, clT=clT, biasf=biasf, lnw=lnw, ident=ident)
    in_maps = []
    for i in range(N_CORES):
        xhT = np.ascontiguousarray(xh2[i * M:(i + 1) * M].T)   # (4096, 1024) bf16
        xlT = np.ascontiguousarray(xl2[i * M:(i + 1) * M].T)
        in_maps.append(dict(xhT=xhT, xlT=xlT, **common))
    return in_maps


def kernel(x, codebooks, bias, ln_weight, codes, centroids, _trace=False):
    global _PROG
    if _PROG is None:
        _PROG = _build_program()
    in_maps = _prep_in_maps(x, codebooks, bias, ln_weight, codes, centroids)
    kr = run_bass_kernel_spmd(_PROG, in_maps, list(range(N_CORES)), trace=_trace)
    y = np.concatenate([np.asarray(kr.results[i]["y"]) for i in range(N_CORES)],
                       axis=0)
    # global cmask = OR over cores' local rows; zero masked 128-col blocks
    cm = np.stack([np.asarray(kr.results[i]["cmrow"]).reshape(NCL)
                   for i in range(N_CORES)]).max(axis=0)
    y[:, np.repeat(cm < 0.5, CLS)] = 0.0
    out = y.reshape(B, S, OUT_F).astype(np.float32)
    if _trace:
        return out, kr
    return out
